# revision 43
# baseline (speedup 1.0000x reference)
"""Fused causal multi-head self-attention (pre-LayerNorm) on 8 TRN2 NeuronCores.

Problem: X[2,2048,1024] -> LN -> QKV (16 heads, dh=64) -> causal softmax
attention -> output projection.

Sharding: core c handles batch b = c//4 and head group g = c%4 (4 heads).
Each core computes LN(X_b) (duplicated, cheap), Q/K/V for its 4 heads,
causal attention, and a partial output projection against its 256 rows of
Wo. The host sums the 4 partial outputs per batch (the all-reduce of the
row-sharded projection) and transposes.

Host-side algebraic folds (all O(D^2), negligible vs device work):
  - LN affine:  Xn = z*ln_w + ln_b  with z=(x-mu)*rstd, so
      Xn @ W + b == z @ (ln_w[:,None]*W) + (ln_b @ W + b)
  - score scale 1/sqrt(dh) folded into Wq/bq.
  - column sums of the effective weights negated (csqn etc.) so the LN
    mean-correction is a single fused scalar_tensor_tensor:
      inner = Qraw + mu*(-csq);  Q = inner * rstd  (2 DVE ops, not 3).

Device layout notes (per core):
  - xt:  X_b^T as bf16, [D=1024, S=2048]; contraction dims live on SBUF
    partitions so no on-device transposes are needed anywhere.
  - LN stats via ones-vector matmuls (reduce over partitions on the PE).
  - Q,K are produced transposed [head_pair*128, S]; scores are computed
    transposed St[k,q] so softmax's k-reduction is a PE reduction: the AV
    matmul uses lhsT=[V|1] whose last column yields the softmax
    denominator for free. exp() runs without max-subtraction (scores are
    bounded ~|17| here, safe in f32/bf16).
  - The two heads of a pair write scores into adjacent PSUM banks of one
    [128,2,512] tile; ONE strided activation computes both heads' exp
    (halves ACT instruction count).
  - Softmax 1/denom via ACT: recip = Exp(-Ln(denom)) — both functions
    live in activation-table set 6, so no table reloads and no slow DVE
    RECIPROCAL ops.
  - Output projection computed transposed outT = Wo_slice^T @ AVt into a
    per-chunk [128, 8, 512] buffer, stored with one DMA per chunk.
"""

import os
import numpy as np
import ml_dtypes

S = 2048
D = 1024
DH = 64
H_PER_CORE = 4
HD = H_PER_CORE * DH  # 256
NQ = S // 512  # 4 q-chunks of 512
ND = D // 128  # 8 d-tiles
NS = S // 128  # 16 s/k-tiles
EPS = 1e-4

_CACHE = {}
LAST_RESULT = None  # BassKernelResults of the most recent run (for test harnesses)


def _build_nc(has_bias: bool, bench_iters: int = 1):
    import concourse.bass as bass
    import concourse.mybir as mybir
    import concourse.tile as tile
    from concourse import bacc
    from contextlib import ExitStack, nullcontext

    f32 = mybir.dt.float32
    bf16 = mybir.dt.bfloat16
    MULT = mybir.AluOpType.mult
    SUB = mybir.AluOpType.subtract
    ADD = mybir.AluOpType.add
    EXP = mybir.ActivationFunctionType.Exp
    LN = mybir.ActivationFunctionType.Ln

    nc = bacc.Bacc("TRN2", target_bir_lowering=False, debug=False, num_devices=8)

    xt = nc.dram_tensor("xt", [D, S], bf16, kind="ExternalInput").ap()
    wq = nc.dram_tensor("wq", [D, HD], bf16, kind="ExternalInput").ap()
    wk = nc.dram_tensor("wk", [D, HD], bf16, kind="ExternalInput").ap()
    wv = nc.dram_tensor("wv", [D, HD], bf16, kind="ExternalInput").ap()
    wo = nc.dram_tensor("wo", [HD, D], bf16, kind="ExternalInput").ap()
    # small f32 per-partition constants packed into one tensor/DMA:
    # [bq(2) bk(2) bv(2) bo(8) csqn(2) cskn(2) csvn(256)] = 274 columns
    consts = nc.dram_tensor("consts", [128, 274], f32,
                            kind="ExternalInput").ap()
    mask = nc.dram_tensor("mask", [128, 2, 128], bf16, kind="ExternalInput").ap()
    out = nc.dram_tensor("out", [D, S], f32, kind="ExternalOutput").ap()

    with tile.TileContext(nc) as tc, ExitStack() as ctx:
        const = ctx.enter_context(tc.tile_pool(name="const", bufs=1))
        big = ctx.enter_context(tc.tile_pool(name="big", bufs=1))
        tmp = ctx.enter_context(tc.tile_pool(name="tmp", bufs=2))
        apool = ctx.enter_context(tc.tile_pool(name="apool", bufs=5))
        rpool = ctx.enter_context(tc.tile_pool(name="rpool", bufs=2))
        obuf = ctx.enter_context(tc.tile_pool(name="obuf", bufs=2))
        dpool = ctx.enter_context(tc.tile_pool(name="dram", bufs=2, space="DRAM"))
        # PSUM budget is 8 banks: work(2) scp(2x2) av0(1) av1(1)
        ps_work = ctx.enter_context(
            tc.tile_pool(name="ps_work", bufs=2, space="PSUM"))
        ps_sc = ctx.enter_context(
            tc.tile_pool(name="ps_sc", bufs=2, space="PSUM"))
        ps_av = ctx.enter_context(
            tc.tile_pool(name="ps_av", bufs=1, space="PSUM"))

        # ---- constants / weights
        wq_sb = const.tile([128, ND, HD], bf16, tag="wq")
        wk_sb = const.tile([128, ND, HD], bf16, tag="wk")
        wv_sb = const.tile([128, ND, HD], bf16, tag="wv")
        wo_sb = const.tile([128, 2, D], bf16, tag="wo")
        consts_sb = const.tile([128, 274], f32, tag="consts")
        bq_sb = consts_sb[:, 0:2]
        bk_sb = consts_sb[:, 2:4]
        bv_sb = consts_sb[:, 4:6]
        bo_sb = consts_sb[:, 6:14]
        csqn_sb = consts_sb[:, 14:16]
        cskn_sb = consts_sb[:, 16:18]
        csvn_sb = consts_sb[:, 18:274]
        # additive causal mask pair: tri[q',k] = -1e4 if q' < k (strictly
        # above the block diagonal), plus a 128x128 identity. One PE
        # matmul (lhsT=tri, rhs=identity) accumulates tri^T onto a
        # diagonal score block, replacing the post-exp DVE mask multiply.
        tri_sb = const.tile([128, 2, 128], bf16, tag="tri")
        ones_sb = const.tile([128, 1], bf16, tag="ones")
        onef_sb = const.tile([1, 1], f32, tag="onef")
        epsf_sb = const.tile([1, 1], f32, tag="epsf")
        nc.vector.memset(epsf_sb, EPS)

        xt_sb = big.tile([128, ND, S], bf16, tag="xt")

        # chunk-0 xt columns land first; chunks 1-3 follow as wide loads.
        # All xt on the sync DMA queue, weights on the scalar queue.
        for dt in range(ND):
            nc.sync.dma_start(xt_sb[:, dt, 0:512],
                              xt[dt * 128:(dt + 1) * 128, 0:512])
        for dt in range(ND):
            nc.sync.dma_start(xt_sb[:, dt, 512:S],
                              xt[dt * 128:(dt + 1) * 128, 512:S])
        nc.scalar.dma_start(consts_sb, consts)
        nc.scalar.dma_start(wq_sb, wq.rearrange("(t p) n -> p t n", p=128))
        nc.scalar.dma_start(wk_sb, wk.rearrange("(t p) n -> p t n", p=128))
        nc.scalar.dma_start(wv_sb, wv.rearrange("(t p) n -> p t n", p=128))
        nc.scalar.dma_start(wo_sb, wo.rearrange("(t p) n -> p t n", p=128))
        nc.scalar.dma_start(tri_sb, mask)
        nc.vector.memset(ones_sb, 1.0)
        nc.vector.memset(onef_sb, 1.0)
        # Pre-load the combined Ln+Exp activation-table set so the
        # compiler's table-load pass sees every activation covered by one
        # resident set.
        _ldset = mybir.InstLoadActFuncSet(
            name=nc.get_next_instruction_name(), ins=[], outs=[],
            act_func_set_id=6)
        nc.scalar.add_instruction(_ldset)

        # ---- persistent activations
        qt_sb = big.tile([128, 2, S], bf16, tag="qt")
        kt_sb = big.tile([128, 2, S], bf16, tag="kt")
        v_sb = big.tile([128, NS, H_PER_CORE, DH + 1], bf16, tag="v")
        avt_sb = big.tile([128, 2, S], bf16, tag="avt")
        # per-chunk stat rows (disjoint columns per chunk)
        mu_row = big.tile([1, S], f32, tag="murow")
        rstd_row = big.tile([1, S], f32, tag="rstdrow")
        # column layouts of mu and rstd per s-tile (for the V correction)
        mcol_sb = big.tile([128, NS], f32, tag="mcol")
        rcol_sb = big.tile([128, NS], f32, tag="rcol")

        # V's trailing all-ones column (softmax denominator trick)
        nc.vector.memset(v_sb[:, :, :, DH:DH + 1], 1.0)

        def _attention_block(qt, filler, pump):
            """Causal attention for q-chunk qt.

            Engine queues are FIFO, so matmuls emitted after this block
            cannot fill the PE stalls of its ACT-bound exp chain. `filler`
            is an iterator of independent work (next chunk's stats/QKV/V,
            previous chunk's out-projection) pumped between score/AV
            groups to keep the PE warm (HAM) and busy.
            """
            qs = slice(qt * 512, (qt + 1) * 512)
            nkt = 4 * qt + 4
            pump_acc = [0.0]
            for p in range(2):
                # both heads' AV accumulators in one 2-bank tile so the two
                # softmax-denominator rows form one contiguous [1,1024] AP
                avp = ps_av.tile([DH + 1, 2, 512], f32, tag="avp")
                av0 = avp[:, 0, :]
                av1 = avp[:, 1, :]
                for kt in range(nkt):
                    ks = slice(kt * 128, (kt + 1) * 128)
                    # Diagonal-region k-tiles (jj>=0) contribute nothing to
                    # q-columns left of jj*128: slice scores/exp/AV to the
                    # valid region only.
                    jj = kt - 4 * qt
                    vs = max(0, jj) * 128
                    vls = slice(vs, 512)
                    qv = qt_sb[:, p, qt * 512 + vs:(qt + 1) * 512]
                    scp = ps_sc.tile([128, 2, 512], f32, tag="scp")
                    # two heads' score matmuls target disjoint PE row groups
                    # (lhsT base partitions 0 / 64) -> they run concurrently
                    nc.tensor.matmul(scp[:, 0, vls], kt_sb[0:64, p, ks],
                                     qv[0:64, :], start=True, stop=(jj < 0))
                    nc.tensor.matmul(scp[:, 1, vls], kt_sb[64:128, p, ks],
                                     qv[64:128, :], start=True, stop=(jj < 0))
                    if jj >= 0:
                        # accumulate the additive causal triangle onto the
                        # diagonal 128-wide block (PE, pre-exp) instead of
                        # a post-exp DVE multiply on the critical path
                        dsl = slice(jj * 128, (jj + 1) * 128)
                        nc.tensor.matmul(scp[:, 0, dsl], tri_sb[:, 0, :],
                                         tri_sb[:, 1, :],
                                         start=False, stop=True)
                        nc.tensor.matmul(scp[:, 1, dsl], tri_sb[:, 0, :],
                                         tri_sb[:, 1, :],
                                         start=False, stop=True)
                    # One strided exp covers both heads' scores. K was left
                    # un-multiplied by rstd; the per-k rstd factor is
                    # restored here via the activation's per-partition
                    # scale AP: exp(rstd_k * s~) = exp(s_true).
                    ap = apool.tile([128, 2, 512], bf16, tag="a")
                    sc_scale = (1.0 if has_bias
                                else rcol_sb[:, kt:kt + 1])
                    nc.scalar.activation(ap[:, :, vls], scp[:, :, vls], EXP,
                                         scale=sc_scale)
                    nc.tensor.matmul(av0[:, vls], v_sb[:, kt, 2 * p, :],
                                     ap[:, 0, vls],
                                     start=(kt == 0), stop=(kt == nkt - 1))
                    nc.tensor.matmul(av1[:, vls], v_sb[:, kt, 2 * p + 1, :],
                                     ap[:, 1, vls],
                                     start=(kt == 0), stop=(kt == nkt - 1))
                    # pump independent filler work into the engine queues
                    # (fractional rate spreads scarce filler over the
                    # whole chunk instead of exhausting it early)
                    pump_acc[0] += pump
                    while pump_acc[0] >= 1.0:
                        next(filler, None)
                        pump_acc[0] -= 1.0
                # Evacuate raw (unnormalized) AV immediately — the psum
                # pair bank is WAR-blocking the next pair's AV matmuls, so
                # don't hold it through the reciprocal roundtrip.
                lnp = rpool.tile([1, 2, 512], f32, tag="lnp")
                nc.scalar.activation(lnp, avp[DH:DH + 1, :, :], LN)
                for j, av_ps in ((0, av0), (1, av1)):
                    nc.vector.tensor_copy(avt_sb[64 * j:64 * j + 64, p, qs],
                                          av_ps[0:DH, :])
                # softmax 1/denom on ACT: recip = Exp(-Ln(denom)); both in
                # table set 6 so no table reloads, no slow DVE RECIPROCAL.
                # The Ln'd row bounces through DRAM into [8,128] so the
                # exp uses 8 ACT lanes instead of 1.
                ln_dr = dpool.tile([1, 1024], f32, tag="lnd")
                nc.sync.dma_start(ln_dr, lnp.rearrange("p t n -> p (t n)"))
                lnr = rpool.tile([8, 128], f32, tag="lnr")
                nc.sync.dma_start(
                    lnr, ln_dr.rearrange("p (a b) -> (p a) b", a=8))
                recp = rpool.tile([8, 128], f32, tag="recp")
                nc.scalar.activation(recp, lnr, EXP, scale=-1.0)
                rec_dr = dpool.tile([1, 1024], f32, tag="recd")
                nc.sync.dma_start(
                    rec_dr.rearrange("p (a b) -> (p a) b", a=8), recp)
                recb = rpool.tile([128, 512], f32, tag="recb")
                nc.sync.dma_start(recb[0:DH, :],
                                  rec_dr[0:1, 0:512].partition_broadcast(DH))
                nc.sync.dma_start(recb[DH:128, :],
                                  rec_dr[0:1, 512:1024].partition_broadcast(DH))
                # normalize in place once the broadcast lands (off the
                # psum critical path)
                for j in range(2):
                    hrow = slice(64 * j, 64 * j + 64)
                    avd = avt_sb[hrow, p, qs]
                    nc.vector.tensor_tensor(out=avd, in0=avd,
                                            in1=recb[hrow, :], op=MULT)
                    if has_bias:
                        nc.vector.tensor_scalar_add(avd, avd,
                                                    bv_sb[hrow, p:p + 1])
                # bridge the reciprocal roundtrip with extra filler
                for _ in range(max(1, int(pump))):
                    next(filler, None)
        def _gen_outproj(qt):
            """Output projection of chunk qt (emitted as filler under the
            NEXT chunk's attention; depends only on avt of chunk qt)."""
            qs = slice(qt * 512, (qt + 1) * 512)
            ob = obuf.tile([128, ND, 512], f32, tag="ob")
            for ot in range(ND):
                o_ps = ps_work.tile([128, 512], f32, tag="work")
                osl = slice(ot * 128, (ot + 1) * 128)
                for p in range(2):
                    nc.tensor.matmul(o_ps, wo_sb[:, p, osl],
                                     avt_sb[:, p, qs],
                                     start=(p == 0), stop=(p == 1))
                nc.vector.tensor_scalar_add(ob[:, ot, :], o_ps,
                                            bo_sb[:, ot:ot + 1])
                yield
            nc.sync.dma_start(
                out.rearrange("(t p) s -> p t s", p=128)[:, :, qs], ob)

        def _gen_pre(qc):
            """LN stats + QKV + V for chunk qc, yielding at interleave
            points so it can be pumped as PE filler under the previous
            chunk's ACT-bound attention."""
            qs = slice(qc * 512, (qc + 1) * 512)
            # mu then ss through one PSUM bank
            mu_ps = ps_work.tile([1, 512], f32, tag="work")
            for dt in range(ND):
                nc.tensor.matmul(mu_ps, ones_sb, xt_sb[:, dt, qs],
                                 start=(dt == 0), stop=(dt == ND - 1))
                if dt % 2 == 1:
                    yield
            mu_c = mu_row[0:1, qs]
            nc.vector.tensor_scalar_mul(mu_c, mu_ps, 1.0 / D)
            ss_ps = ps_work.tile([1, 512], f32, tag="work")
            # chunk 0's squares on DVE (startup-critical), rest on GPSIMD
            sq_eng = nc.vector if qc == 0 else nc.gpsimd
            for dt in range(ND):
                xsl = xt_sb[:, dt, qs]
                sq = tmp.tile([128, 512], bf16, tag="sq")
                sq_eng.tensor_tensor(out=sq, in0=xsl, in1=xsl, op=MULT)
                nc.tensor.matmul(ss_ps, ones_sb, sq,
                                 start=(dt == 0), stop=(dt == ND - 1))
                if dt % 2 == 1:
                    yield
            msq = tmp.tile([1, 512], f32, tag="msq")
            nc.vector.tensor_tensor(out=msq, in0=mu_c, in1=mu_c, op=MULT)
            var = tmp.tile([1, 512], f32, tag="var")
            nc.vector.scalar_tensor_tensor(out=var, in0=ss_ps,
                                           scalar=1.0 / D, in1=msq,
                                           op0=MULT, op1=SUB)
            # rstd = (var+eps)^-0.5 = exp(-0.5*ln(var+eps)); Ln+Exp share
            # one ACT table set
            lnv = tmp.tile([1, 512], f32, tag="lnv")
            nc.scalar.activation(lnv, var, LN, bias=epsf_sb)
            nc.scalar.activation(rstd_row[0:1, qs], lnv, EXP, scale=-0.5)
            # broadcast [mu; rstd] rows over 128 partitions via DRAM bounce
            # (SBUF DMA sources cannot have a zero partition step)
            mrow_dr = dpool.tile([1, 2, 512], f32, tag="mrowd")
            nc.sync.dma_start(mrow_dr[0:1, 0, :], mu_c)
            nc.sync.dma_start(mrow_dr[0:1, 1, :], rstd_row[0:1, qs])
            stb = rpool.tile([128, 2, 512], f32, tag="stb")  # 0=mu_b 1=rstdb
            nc.sync.dma_start(stb, mrow_dr.partition_broadcast(128))
            yield
            # row->column transpose of mu and rstd via K=1 N=1 matmuls
            cts = slice(4 * qc, 4 * qc + 4)
            colps = ps_work.tile([128, 8], f32, tag="work")
            for i in range(4):
                st_ = slice((4 * qc + i) * 128, (4 * qc + i + 1) * 128)
                nc.tensor.matmul(colps[:, i:i + 1], mu_row[0:1, st_],
                                 onef_sb, start=True, stop=True)
                nc.tensor.matmul(colps[:, 4 + i:5 + i], rstd_row[0:1, st_],
                                 onef_sb, start=True, stop=True)
            nc.vector.tensor_copy(mcol_sb[:, cts], colps[:, 0:4])
            nc.vector.tensor_copy(rcol_sb[:, cts], colps[:, 4:8])
            yield

            for p in range(2):
                hp = slice(p * 128, (p + 1) * 128)
                for w_sb, csn_sb, b_sb, dst in (
                        (wq_sb, csqn_sb, bq_sb, qt_sb),
                        (wk_sb, cskn_sb, bk_sb, kt_sb)):
                    ps = ps_work.tile([128, 512], f32, tag="work")
                    for dt in range(ND):
                        nc.tensor.matmul(ps, w_sb[:, dt, hp],
                                         xt_sb[:, dt, qs],
                                         start=(dt == 0),
                                         stop=(dt == ND - 1))
                        if dt % 3 == 2:
                            yield
                    # inner = Qraw + mu_b*(-cs);  Q = inner * rstdb.
                    # K (bias-free) skips the rstd multiply entirely: its
                    # per-k rstd factor rides the attention exp's scale AP.
                    d = dst[:, p, qs]
                    if dst is kt_sb and not has_bias:
                        nc.vector.scalar_tensor_tensor(
                            out=d, in0=stb[:, 0, :],
                            scalar=csn_sb[:, p:p + 1], in1=ps,
                            op0=MULT, op1=ADD)
                        yield
                        continue
                    t1 = tmp.tile([128, 512], f32, tag="qk_t1")
                    nc.vector.scalar_tensor_tensor(
                        out=t1, in0=stb[:, 0, :],
                        scalar=csn_sb[:, p:p + 1], in1=ps,
                        op0=MULT, op1=ADD)
                    nc.vector.tensor_tensor(
                        out=d, in0=t1,
                        in1=stb[:, 1, :], op=MULT)
                    if has_bias:
                        nc.vector.tensor_scalar_add(d, d, b_sb[:, p:p + 1])
                    yield
            for st in range(4 * qc, 4 * qc + 4):
                ss_ = slice(st * 128, (st + 1) * 128)
                v_ps = ps_work.tile([128, HD], f32, tag="work")
                for dt in range(ND):
                    nc.tensor.matmul(v_ps, xt_sb[:, dt, ss_],
                                     wv_sb[:, dt, :],
                                     start=(dt == 0), stop=(dt == ND - 1))
                    if dt % 4 == 3:
                        yield
                # inner = Vraw + mu_col*(-csv);  V = inner * rstd_col
                tv1 = tmp.tile([128, HD], f32, tag="v_t1")
                nc.vector.scalar_tensor_tensor(
                    out=tv1, in0=csvn_sb, scalar=mcol_sb[:, st:st + 1],
                    in1=v_ps, op0=MULT, op1=ADD)
                nc.vector.tensor_scalar_mul(
                    v_sb[:, st, :, 0:DH],
                    tv1.rearrange("p (h d) -> p h d", h=H_PER_CORE),
                    rcol_sb[:, st:st + 1])
                yield

        def _drain(it):
            for _ in it:
                pass

        # Chunk pipeline: chunk 0's stats/QKV run up front; each chunk's
        # attention then pumps the NEXT chunk's stats/QKV/V and the
        # PREVIOUS chunk's out-projection as PE filler between its
        # ACT-bound exp groups.
        # bench_iters > 1 wraps the compute in a hardware loop so the
        # differential harness gets a k-times-larger device-time signal
        # (input loads stay outside the loop).
        import itertools as _it
        loop_cm = (tc.For_i(0, bench_iters, 1) if bench_iters > 1
                   else nullcontext())
        with loop_cm:
            _drain(_gen_pre(0))
            # pump rates ~= available filler units / attention slots so
            # filler spreads across each chunk's whole attention span
            pumps = {0: 3.75, 1: 2.4, 2: 1.6, 3: 0.25}
            for qc in range(NQ):
                filler = []
                if qc > 0:
                    filler.append(_gen_outproj(qc - 1))
                if qc + 1 < NQ:
                    filler.append(_gen_pre(qc + 1))
                filler = _it.chain(*filler)
                _attention_block(qc, filler, pumps[qc])
                _drain(filler)
            _drain(_gen_outproj(NQ - 1))

    nc.compile()
    return nc


def _prep_in_maps(inputs, has_bias=False):
    bf = ml_dtypes.bfloat16
    X = np.asarray(inputs["X"], np.float32)
    ln_w = np.asarray(inputs["ln_w"], np.float32)
    ln_b = np.asarray(inputs["ln_b"], np.float32)
    Wq = np.asarray(inputs["Wq"], np.float32)
    Wk = np.asarray(inputs["Wk"], np.float32)
    Wv = np.asarray(inputs["Wv"], np.float32)
    Wo = np.asarray(inputs["Wo"], np.float32)
    bq = np.asarray(inputs["bq"], np.float32)
    bk = np.asarray(inputs["bk"], np.float32)
    bv = np.asarray(inputs["bv"], np.float32)
    bo = np.asarray(inputs["bo"], np.float32)

    scale = 1.0 / np.sqrt(DH).astype(np.float32)
    Wq_eff = ln_w[:, None] * Wq * scale
    bq_eff = (ln_b @ Wq + bq) * scale
    Wk_eff = ln_w[:, None] * Wk
    bk_eff = ln_b @ Wk + bk
    Wv_eff = ln_w[:, None] * Wv
    bv_eff = ln_b @ Wv + bv

    # additive causal mask pair: tri[q',k] = -1e4 where q' < k, and the
    # 128x128 identity used as the mask-matmul's moving operand
    ii = np.arange(128)
    tri = np.where(ii[:, None] < ii[None, :], -1e4, 0.0).astype(np.float32)
    mask = np.stack([tri, np.eye(128, dtype=np.float32)], axis=1).astype(bf)

    # negated column sums of the (bf16-rounded) effective weights, for the
    # fused post-matmul mean correction: inner = raw + mu*(-colsum)
    csqn_full = -Wq_eff.astype(bf).astype(np.float32).sum(axis=0)
    cskn_full = -Wk_eff.astype(bf).astype(np.float32).sum(axis=0)
    csvn_full = -Wv_eff.astype(bf).astype(np.float32).sum(axis=0)

    in_maps = []
    for c in range(8):
        b, g = c // 4, c % 4
        hs = slice(g * HD, (g + 1) * HD)
        in_maps.append({
            "xt": np.ascontiguousarray(X[b].T).astype(bf),
            "wq": Wq_eff[:, hs].astype(bf),
            "wk": Wk_eff[:, hs].astype(bf),
            "wv": Wv_eff[:, hs].astype(bf),
            "wo": np.ascontiguousarray(Wo[hs, :]).astype(bf),
            "consts": np.concatenate([
                bq_eff[hs].reshape(2, 128).T,
                bk_eff[hs].reshape(2, 128).T,
                bv_eff[hs].reshape(2, 128).T,
                (bo.reshape(ND, 128).T if g == 0
                 else np.zeros((128, ND), np.float32)),
                csqn_full[hs].reshape(2, 128).T,
                cskn_full[hs].reshape(2, 128).T,
                np.tile(csvn_full[hs][None, :], (128, 1)),
            ], axis=1).astype(np.float32),
            "mask": mask,
        })
    return in_maps


def _has_bias(inputs):
    return any(
        np.any(np.asarray(inputs[k], np.float32) != 0.0)
        for k in ("bq", "bk", "bv", "bo", "ln_b"))


def kernel(**inputs) -> np.ndarray:
    global LAST_RESULT
    from concourse.bass_utils import run_bass_kernel_spmd

    has_bias = _has_bias(inputs)
    key = ("nc", has_bias)
    if key not in _CACHE:
        _CACHE[key] = _build_nc(has_bias)
    nc = _CACHE[key]
    _CACHE["nc"] = nc  # most-recent, for test harnesses

    in_maps = _prep_in_maps(inputs, has_bias)
    import time as _time
    t0 = _time.time()
    res = run_bass_kernel_spmd(
        nc, in_maps, core_ids=list(range(8)),
        trace=bool(int(os.environ.get("KERNEL_TRACE", "0"))),
    )
    _CACHE["exec_wall_s"] = _time.time() - t0
    LAST_RESULT = res
    outs = [r["out"] for r in res.results]
    full = np.stack([
        (outs[0] + outs[1] + outs[2] + outs[3]).T,
        (outs[4] + outs[5] + outs[6] + outs[7]).T,
    ]).astype(np.float32)
    return full
